# revision 1
# baseline (speedup 1.0000x reference)
"""CapsNet feature extractor on 8 Trainium2 NeuronCores (Bass/Tile). v2

Sharding: phase A (conv1 -> bn/relu -> pconv -> squash) is data-parallel over
batch (8 images/core). An AllToAll reshards u from batch-sharded to
routes-sharded (576 routes/core; core j owns capsule type j). Phase B (u_hat,
dynamic routing) is routes-sharded with an AllReduce on s each iteration; the
final FC stack is computed redundantly on every core; core 0's output is
returned.

v2 changes vs v1:
- conv1 uses a single 108-descriptor full-row input load + kx-batched
  tile_position matmuls (kills the 3MB/image im2col HBM blowup).
- conv1 output stored even/odd-column split (h_eo) so pconv matmul rhs APs
  are stride-1.
- u is transposed on-chip (DVE 32x32 block transpose) before staging, and the
  AllToAll buffers are laid out [dst, kq2, c, img, g] so both the send and
  the post-collective load move >=288B contiguous runs (v1's load moved 2-byte
  runs: ~1.2M descriptors).
- u_hat writeback batched to one DMA per 8-route group (576 -> 72 DMAs).
- W_caps host prep permuted to match the new route->(kq2,g) mapping.
"""
import os
import sys
sys.path.insert(0, '/opt/trn_rl_repo')
import numpy as np
from contextlib import ExitStack

import concourse.bass as bass
import concourse.bacc as bacc
import concourse.mybir as mybir
from concourse import tile
from concourse.bass_utils import run_bass_kernel_spmd

dt = mybir.dt
AF = mybir.ActivationFunctionType
ALU = mybir.AluOpType
AX = mybir.AxisListType

N_CORES = 8
B = 64; IN_C = 3; IMG = 64
NCAP = 8; PC = 32; ND = 10; DC = 16; FEAT = 128
C1 = 56
PR = 24
ROUTES = 4608
BPC = B // N_CORES
RPC = ROUTES // N_CORES
NO = ND * DC                # 160
NG = RPC // 4               # 144
NT = 5                      # u_hat partition tiles (4 x 128 + 1 x 64)
F32, BF16, F32R = dt.float32, dt.bfloat16, dt.float32r


def ap(t, offset, dims):
    """Manual access pattern; dims = [[step, count], ...] outer->inner, elems."""
    if isinstance(t, bass.AP):
        return bass.AP(tensor=t.tensor, offset=t.offset + offset,
                       ap=[list(d) for d in dims])
    return bass.AP(tensor=t, offset=offset, ap=[list(d) for d in dims])


def fap(tl, p0, pcnt, off, dims, pstep_mult=1):
    """AP into an SBUF tile AP `tl`: partition run [p0, p0+pcnt) with partition
    step `pstep_mult` rows, then free dims (offset `off` within partition)."""
    pstep = tl.ap[0][0]
    return bass.AP(tensor=tl.tensor, offset=tl.offset + p0*pstep + off,
                   ap=[[pstep*pstep_mult, pcnt]] + [list(d) for d in dims])


def xap(tl, p0, off, dims):
    """AP into an SBUF tile with explicit (possibly partition-crossing) dims:
    entries of `dims` may be ('p', mult, count) for partition-stepping levels
    or [step, count] for free-dim levels."""
    pstep = tl.ap[0][0]
    out = []
    for d in dims:
        if isinstance(d, tuple) and d[0] == 'p':
            out.append([pstep * d[1], d[2]])
        else:
            out.append([d[0], d[1]])
    return bass.AP(tensor=tl.tensor, offset=tl.offset + p0*pstep + off, ap=out)


def build(debug=False):
    nc = bacc.Bacc("TRN2", target_bir_lowering=False, debug=False,
                   num_devices=N_CORES)

    xs = nc.declare_dram_parameter("xs", [BPC*IN_C*IMG*IMG + 3], F32R, isOutput=False)
    w1k = nc.declare_dram_parameter("w1k", [128, 3*256], F32R, isOutput=False)
    b1 = nc.declare_dram_parameter("b1", [256], F32, isOutput=False)
    w2 = nc.declare_dram_parameter("w2", [128, 2*81*2*128], BF16, isOutput=False)
    b2 = nc.declare_dram_parameter("b2", [256], F32, isOutput=False)
    wk = nc.declare_dram_parameter("wk", [128, NG*NO], BF16, isOutput=False)
    fc1t = nc.declare_dram_parameter("fc1t", [161, 512], F32R, isOutput=False)
    fc2t = nc.declare_dram_parameter("fc2t", [513, 256], F32R, isOutput=False)
    fc3t = nc.declare_dram_parameter("fc3t", [257, 128], F32R, isOutput=False)
    ident = nc.declare_dram_parameter("ident", [128, 128], F32R, isOutput=False)
    onesd = nc.declare_dram_parameter("onesd", [128, 64], F32R, isOutput=False)
    out = nc.declare_dram_parameter("out", [B, FEAT], F32, isOutput=True)

    # A2A staging: flat [dst, kq2*c*img*g]
    u_send = nc.dram_tensor("u_send", [N_CORES, 4*PC*BPC*NG], BF16)
    u_recv = nc.dram_tensor("u_recv", [N_CORES, 4*PC*BPC*NG], BF16)
    s_send = nc.dram_tensor("s_send", [3, B*NO], F32)
    s_recv = nc.dram_tensor("s_recv", [3, B*NO], F32)
    v_stage = nc.dram_tensor("v_stage", [ND, B*DC], F32R)
    groups = [list(range(N_CORES))]

    with tile.TileContext(nc) as tc, ExitStack() as top:
        consts = top.enter_context(tc.tile_pool(name="consts", bufs=1))

        b1t = consts.tile([128, 2], F32)
        nc.sync.dma_start(b1t[:], ap(b1, 0, [[1, 128], [128, 2]]))
        b2t = consts.tile([128, 2], F32)
        nc.sync.dma_start(b2t[:], ap(b2, 0, [[1, 128], [128, 2]]))
        onesb = consts.tile([128, 1], BF16)
        nc.vector.memset(onesb[:], 1.0)
        onesrow = consts.tile([1, 128], BF16)
        nc.vector.memset(onesrow[:], 1.0)
        onesf = consts.tile([128, 64], F32R)
        nc.sync.dma_start(onesf[:], onesd[:])
        identT = consts.tile([128, 128], F32R)
        nc.sync.dma_start(identT[:], ident[:])
        zt = consts.tile([128, 1], F32)
        nc.vector.memset(zt[:], 0.0)
        epst = consts.tile([128, 1], F32)
        nc.vector.memset(epst[:], 1e-8)

        # ============ PHASE A ============
        with tc.tile_pool(name="pha", bufs=1) as pha, \
             tc.tile_pool(name="img", bufs=2) as ipool, \
             tc.tile_pool(name="psumA", bufs=4, space="PSUM") as psumA, \
             tc.tile_pool(name="psumP", bufs=2, space="PSUM") as psumP:
            w1sb = pha.tile([128, 3*256], F32R)
            nc.sync.dma_start(w1sb[:], w1k[:])
            w2sb = pha.tile([128, 2*81*2*128], BF16)
            nc.sync.dma_start(w2sb[:], w2[:])

            for img in range(BPC):
                # --- input rows: partition 32t + ic*9 + ky holds
                #     x[ic, ky:ky+56, 0:64] (3584 elems), replicated t=0..3 ---
                # block t (partitions 27t + 9ic + ky) holds x[ic, ky:ky+56, t:t+64]
                # (kx%4 shift baked into the data) so one plain K=108 matmul
                # covers kx = 4*bt + t for all 4 t's at once.
                xrows = ipool.tile([128, C1*IMG], F32R, tag="xrows")
                for t in range(4):
                    nc.sync.dma_start(
                        xap(xrows, 27*t, 0, [('p', 1, 27), [1, C1*IMG]]),
                        ap(xs, img*IN_C*IMG*IMG + t,
                           [[IMG*IMG, 3], [IMG, 9], [1, C1*IMG]]))

                # --- conv1 (+bn+relu) -> h_eo[p, e*3136 + och*1568 + y*28 + x'] ---
                h_eo = ipool.tile([128, 2*C1*C1], BF16, tag="heo")
                for och in range(2):
                    for oy0 in range(0, 7, 2):
                        noy = min(2, 7 - oy0)
                        pscs = [psumA.tile([128, 448], F32, tag="psc1",
                                           name=f"psc_{och}_{oy0}_{q}")
                                for q in range(noy)]
                        for bt in range(3):
                            lhsT = w1sb[0:108, bt*256 + och*128: bt*256 + (och+1)*128]
                            for q in range(noy):
                                oyc = oy0 + q
                                nc.tensor.matmul(
                                    pscs[q][:],
                                    lhsT,
                                    fap(xrows, 0, 108, oyc*8*IMG + 4*bt,
                                        [[IMG, 8], [1, C1]]),
                                    start=(bt == 0), stop=(bt == 2))
                        for q in range(noy):
                            oyc = oy0 + q
                            nc.scalar.activation(
                                fap(h_eo, 0, 128, och*1568 + oyc*224,
                                    [[C1*C1, 2], [28, 8], [1, 28]]),
                                fap(pscs[q], 0, 128, 0,
                                    [[1, 2], [C1, 8], [2, 28]]),
                                AF.Relu, bias=b1t[:, och:och+1])

                # --- pconv (stride 2) -> pst[p, och*576 + oy*24 + ox] ---
                pst = ipool.tile([128, 2*PR*PR], F32, tag="pst")
                for och in range(2):
                    psA = psumP.tile([128, 288], F32, tag="psA")
                    psB = psumP.tile([128, 288], F32, tag="psB")
                    for ich in range(2):
                        for ky in range(9):
                            for kx in range(9):
                                widx = ((ich*81 + ky*9 + kx)*2 + och)*128
                                lhsT = w2sb[:, widx:widx+128]
                                first = (ich == 0 and ky == 0 and kx == 0)
                                last = (ich == 1 and ky == 8 and kx == 8)
                                e, tt = kx % 2, kx // 2
                                base = e*C1*C1 + ich*1568 + ky*28 + tt
                                nc.tensor.matmul(
                                    psA[:], lhsT,
                                    fap(h_eo, 0, 128, base, [[56, 12], [1, 24]]),
                                    start=first, stop=last)
                                nc.tensor.matmul(
                                    psB[:], lhsT,
                                    fap(h_eo, 0, 128, base + 672, [[56, 12], [1, 24]]),
                                    start=first, stop=last)
                    nc.scalar.activation(pst[:, och*576:och*576+288], psA[:],
                                         AF.Identity, bias=b2t[:, och:och+1])
                    nc.scalar.activation(pst[:, och*576+288:och*576+576], psB[:],
                                         AF.Identity, bias=b2t[:, och:och+1])

                # --- squash over c = q%32 (groups of 32 in free dim) ---
                p2 = ipool.tile([128, 2*PR*PR], F32, tag="p2")
                nc.vector.tensor_mul(p2[:], pst[:], pst[:])
                sn = ipool.tile([128, 36], F32, tag="sn")
                nc.vector.tensor_reduce(sn[:], fap(p2, 0, 128, 0, [[32, 36], [1, 32]]),
                                        AX.X, ALU.add)
                sn1 = ipool.tile([128, 36], F32, tag="sn1")
                nc.vector.tensor_scalar_add(sn1[:], sn[:], 1.0)
                rde = ipool.tile([128, 36], F32, tag="rde")
                nc.vector.reciprocal(rde[:], sn1[:])
                sqr = ipool.tile([128, 36], F32, tag="sqr")
                nc.scalar.activation(sqr[:], sn[:], AF.Sqrt, bias=epst[:, :])
                rsq = ipool.tile([128, 36], F32, tag="rsq")
                nc.vector.reciprocal(rsq[:], sqr[:])
                scl = ipool.tile([128, 36], F32, tag="scl")
                nc.vector.tensor_mul(scl[:], sn[:], rde[:])
                nc.vector.tensor_mul(scl[:], scl[:], rsq[:])
                usq = ipool.tile([128, 2*PR*PR], BF16, tag="usq")
                nc.vector.tensor_mul(usq[:], pst[:],
                                     fap(scl, 0, 128, 0, [[1, 36], [0, 32]]))

                # --- 32x32 block transpose: T[32kq + c, och*576 + qb*32 + pc'] ---
                T_img = ipool.tile([128, 2*PR*PR], BF16, tag="timg")
                nc.vector.transpose(T_img[:], usq[:])

                # --- stage to u_send[j, kq2, c, img, g] (128 desc x 288B) ---
                for j in range(N_CORES):
                    och, kq = j // 4, j % 4
                    nc.sync.dma_start(
                        ap(u_send, j*(4*PC*BPC*NG) + img*NG,
                           [[BPC*NG, 32], [PC*BPC*NG, 4], [1, NG]]),
                        fap(T_img, 32*kq, 32, och*576, [[NG, 4], [1, NG]]))

        nc.gpsimd.collective_compute("AllToAll", ALU.bypass, replica_groups=groups,
                                     ins=[u_send[:]], outs=[u_recv[:]])

        # ============ PHASE B: u_hat ============
        uhp = top.enter_context(tc.tile_pool(name="uhp", bufs=1))
        uh = [uhp.tile([128, B*NO], BF16, name=f"uh{t}", tag=f"uh{t}")
              for t in range(NT)]
        nc.vector.memset(uh[4][64:128, :], 0.0)

        with tc.tile_pool(name="utp", bufs=1) as utp, \
             tc.tile_pool(name="wstr", bufs=2) as wstr, \
             tc.tile_pool(name="stgp", bufs=4) as stgp, \
             tc.tile_pool(name="psumB", bufs=2, space="PSUM") as psumB:
            # u_all[p=(kq2,c), src*1152 + img*144 + g]
            u_all = utp.tile([128, N_CORES*BPC*NG], BF16)
            for src in range(N_CORES):
                nc.sync.dma_start(
                    u_all[:, src*BPC*NG:(src+1)*BPC*NG],
                    ap(u_recv, src*(4*PC*BPC*NG), [[BPC*NG, 128], [1, BPC*NG]]))

            wchunk = None
            for gp in range(NG // 2):
                ck = (2*gp) // 16
                if (2*gp) % 16 == 0:
                    wchunk = wstr.tile([128, 16*NO], BF16, tag="wchunk")
                    nc.sync.dma_start(wchunk[:], wk[:, ck*16*NO:(ck+1)*16*NO])
                ps = psumB.tile([128, 2048], F32, tag="psuh")
                for rt in range(4):
                    for ct, g in ((0, 2*gp), (1, 2*gp+1)):
                        nc.tensor.matmul(
                            ps[64*ct:64*ct+64, rt*512:rt*512+160],
                            fap(u_all, 32*rt, 32, g, [[BPC*NG, 8], [NG, 8]]),
                            wchunk[32*rt:32*rt+32, (g % 16)*NO:(g % 16 + 1)*NO],
                            start=True, stop=True, tile_position=(32*rt, 64*ct))
                stgt = stgp.tile([128, 640], BF16, tag="stgt")
                pview = fap(ps, 0, 128, 0, [[512, 4], [1, 160]])
                if gp % 2 == 0:
                    nc.scalar.activation(stgt[:], pview, AF.Identity, bias=zt[:, :])
                else:
                    nc.vector.tensor_copy(stgt[:], pview)
                # rows r = 8gp + 4ct + rt of uh tile t (src partition-outer only)
                for rt in range(4):
                    for ct in range(2):
                        r = 8*gp + 4*ct + rt
                        t0, pr_ = r // 128, r % 128
                        nc.sync.dma_start(
                            uh[t0][pr_:pr_+1, :],
                            stgt[64*ct:64*ct+64, rt*160:(rt+1)*160])

        # ============ routing ============
        rp = top.enter_context(tc.tile_pool(name="rp", bufs=1))
        logits = rp.tile([128, NT*B*ND], F32)
        nc.vector.memset(logits[:], 0.0)
        c_t = rp.tile([128, NT*B*ND], BF16)
        s_sb = rp.tile([10, B*DC], F32)
        v_sb = rp.tile([10, B*DC], F32)
        vb16 = rp.tile([10, B*DC], BF16)
        sq2 = rp.tile([10, B*DC], F32)
        snv = rp.tile([10, B], F32)
        snv1 = rp.tile([10, B], F32)
        rdev = rp.tile([10, B], F32)
        sqv = rp.tile([10, B], F32)
        rsqv = rp.tile([10, B], F32)
        sclv = rp.tile([10, B], F32)
        sume = rp.tile([128, NT*B], F32)
        rece = rp.tile([128, NT*B], F32)

        with tc.tile_pool(name="agg", bufs=1) as agg, \
             tc.tile_pool(name="m2cp", bufs=3) as m2cp, \
             tc.tile_pool(name="sstg", bufs=3) as sstg, \
             tc.tile_pool(name="psumR", bufs=2, space="PSUM") as psumR:
            m2b = agg.tile([128, B*NO], BF16)
            l1t = agg.tile([128, B*NO//2], BF16)
            v_rep = agg.tile([128, B*NO], BF16)

            for it in range(3):
                # ---- partial s = sum over local routes of c * u_hat ----
                for chk in range(22):
                    b0 = chk*3
                    nb = min(3, B - b0)
                    w = nb*NO
                    pss = psumR.tile([128, 512], F32, tag="pss")
                    for t in range(NT):
                        kk = 128 if t < 4 else 64
                        if it == 0:
                            rhs = uh[t][0:kk, b0*NO:b0*NO + w]
                        else:
                            m2c = m2cp.tile([128, 512], BF16, tag="m2c")
                            nc.vector.tensor_mul(
                                m2c[0:kk, 0:w], uh[t][0:kk, b0*NO:b0*NO + w],
                                fap(c_t, 0, kk, t*B*ND + b0*ND,
                                    [[ND, nb], [1, ND], [0, DC]]))
                            rhs = m2c[0:kk, 0:w]
                        nc.tensor.matmul(pss[0:1, 0:w], onesb[0:kk, :], rhs,
                                         start=(t == 0), stop=(t == 4))
                    sst = sstg.tile([1, 512], F32, tag="sst")
                    nc.scalar.activation(sst[0:1, 0:w], pss[0:1, 0:w],
                                         AF.Identity, bias=zt[0:1, :],
                                         scale=(0.1 if it == 0 else 1.0))
                    nc.sync.dma_start(ap(s_send, it*B*NO + b0*NO, [[1, w]]),
                                      sst[0:1, 0:w])
                nc.gpsimd.collective_compute(
                    "AllReduce", ALU.add, replica_groups=groups,
                    ins=[ap(s_send, it*B*NO, [[1, B*NO]])],
                    outs=[ap(s_recv, it*B*NO, [[1, B*NO]])])

                # ---- v = squash(s) in [10p=n, (b, o)] ----
                nc.sync.dma_start(s_sb[:],
                                  ap(s_recv, it*B*NO, [[DC, ND], [NO, B], [1, DC]]))
                nc.vector.tensor_mul(sq2[:], s_sb[:], s_sb[:])
                nc.vector.tensor_reduce(snv[:], fap(sq2, 0, 10, 0, [[DC, B], [1, DC]]),
                                        AX.X, ALU.add)
                nc.vector.tensor_scalar_add(snv1[:], snv[:], 1.0)
                nc.vector.reciprocal(rdev[:], snv1[:])
                nc.scalar.activation(sqv[:], snv[:], AF.Sqrt, bias=epst[0:10, :])
                nc.vector.reciprocal(rsqv[:], sqv[:])
                nc.vector.tensor_mul(sclv[:], snv[:], rdev[:])
                nc.vector.tensor_mul(sclv[:], sclv[:], rsqv[:])
                nc.vector.tensor_mul(v_sb[:], s_sb[:],
                                     fap(sclv, 0, 10, 0, [[1, B], [0, DC]]))
                if it == 2:
                    nc.sync.dma_start(v_stage[:].bitcast(F32), v_sb[:])
                    break
                nc.vector.tensor_copy(vb16[:], v_sb[:])
                for n in range(ND):
                    vrow = m2cp.tile([1, B*DC], BF16, tag="vrow")
                    nc.sync.dma_start(vrow[0:1, :], vb16[n:n+1, :])
                    psv = psumR.tile([128, 1024], F32, tag="psv", bufs=1)
                    nc.tensor.matmul(psv[:, 0:512], onesrow[0:1, :],
                                     vrow[0:1, 0:512], start=True, stop=True)
                    nc.tensor.matmul(psv[:, 512:1024], onesrow[0:1, :],
                                     vrow[0:1, 512:1024], start=True, stop=True)
                    nc.scalar.activation(
                        fap(v_rep, 0, 128, n*DC, [[NO, B], [1, DC]]),
                        psv[:], AF.Identity, bias=zt[:, :])

                # ---- agreement: logits += sum_o u_hat * v ----
                for t in range(NT):
                    nc.vector.tensor_mul(m2b[:], uh[t][:], v_rep[:])
                    nc.vector.tensor_add(
                        l1t[:], fap(m2b, 0, 128, 0, [[DC, B*ND], [1, 8]]),
                        fap(m2b, 0, 128, 8, [[DC, B*ND], [1, 8]]))
                    nc.vector.tensor_add(
                        m2b[:, 0:B*NO//4],
                        fap(l1t, 0, 128, 0, [[8, B*ND], [1, 4]]),
                        fap(l1t, 0, 128, 4, [[8, B*ND], [1, 4]]))
                    nc.vector.tensor_add(
                        l1t[:, 0:B*NO//8],
                        fap(m2b, 0, 128, 0, [[4, B*ND], [1, 2]]),
                        fap(m2b, 0, 128, 2, [[4, B*ND], [1, 2]]))
                    nc.vector.tensor_add(
                        m2b[:, 0:B*ND],
                        fap(l1t, 0, 128, 0, [[2, B*ND], [1, 1]]),
                        fap(l1t, 0, 128, 1, [[2, B*ND], [1, 1]]))
                    nc.vector.tensor_add(
                        logits[:, t*B*ND:(t+1)*B*ND],
                        logits[:, t*B*ND:(t+1)*B*ND],
                        m2b[:, 0:B*ND])

                # ---- c = softmax(logits) over n (no max-subtraction) ----
                eexp = m2b[:].bitcast(F32)          # [128, 5120] f32 view
                nc.scalar.activation(eexp[:, 0:NT*B*ND], logits[:], AF.Exp, bias=zt[:, :])
                nc.vector.tensor_reduce(
                    sume[:], fap(eexp, 0, 128, 0, [[ND, NT*B], [1, ND]]),
                    AX.X, ALU.add)
                nc.vector.reciprocal(rece[:], sume[:])
                nc.vector.tensor_mul(c_t[:], eexp[:, 0:NT*B*ND],
                                     fap(rece, 0, 128, 0, [[1, NT*B], [0, ND]]))

        # ============ FC head (redundant on every core) ============
        with tc.tile_pool(name="fcp", bufs=1) as fcp, \
             tc.tile_pool(name="psumF", bufs=1, space="PSUM") as psumF:
            fta = fcp.tile([128, B], F32R)
            ftb = fcp.tile([128, B], F32R)
            for n in range(ND):
                dstt, p0 = (fta, n*DC) if n < 8 else (ftb, (n-8)*DC)
                nc.sync.dma_start(dstt[p0:p0+DC, :],
                                  ap(v_stage, n*B*DC, [[1, DC], [DC, B]]))
            nc.sync.dma_start(ftb[32:33, :], onesd[0:1, :])

            fc1a = fcp.tile([128, 512], F32R)
            nc.sync.dma_start(fc1a[:], fc1t[0:128, :])
            fc1b = fcp.tile([128, 512], F32R)
            nc.sync.dma_start(fc1b[0:33, :], fc1t[128:161, :])
            pf1 = psumF.tile([64, 512], F32, tag="pf1")
            nc.tensor.matmul(pf1[:], fta[:, 0:64],
                             fc1a[:], start=True, stop=False)
            nc.tensor.matmul(pf1[:], ftb[0:33, 0:64],
                             fc1b[0:33, :], start=False, stop=True)
            f1 = fcp.tile([64, 512], F32R)
            nc.scalar.activation(f1[:], pf1[:], AF.Relu, bias=zt[0:64, :])

            f1T = fcp.tile([128, 4*64], F32R)
            for k in range(4):
                ptr = psumF.tile([128, 64], F32R, tag="ptr", bufs=2)
                nc.tensor.transpose(ptr[:], f1[:, k*128:(k+1)*128], identT[0:64, 0:64])
                nc.scalar.activation(f1T[:, k*64:(k+1)*64], ptr[:], AF.Identity, bias=zt[:, :])

            fc2a = fcp.tile([128, 4*256], F32R)
            nc.sync.dma_start(fc2a[:], ap(fc2t, 0, [[256, 128], [128*256, 4], [1, 256]]))
            fc2b = fcp.tile([1, 256], F32R)
            nc.sync.dma_start(fc2b[:], fc2t[512:513, :])
            pf2 = psumF.tile([64, 256], F32, tag="pf2")
            for k in range(4):
                nc.tensor.matmul(pf2[:], f1T[:, k*64:(k+1)*64],
                                 fc2a[:, k*256:(k+1)*256],
                                 start=(k == 0), stop=False)
            nc.tensor.matmul(pf2[:], onesf[0:1, :],
                             fc2b[:], start=False, stop=True)
            f2 = fcp.tile([64, 256], F32R)
            nc.scalar.activation(f2[:], pf2[:], AF.Relu, bias=zt[0:64, :])

            f2T = fcp.tile([128, 2*64], F32R)
            for k in range(2):
                ptr2 = psumF.tile([128, 64], F32R, tag="ptr", bufs=2)
                nc.tensor.transpose(ptr2[:], f2[:, k*128:(k+1)*128], identT[0:64, 0:64])
                nc.scalar.activation(f2T[:, k*64:(k+1)*64], ptr2[:], AF.Identity, bias=zt[:, :])

            fc3a = fcp.tile([128, 2*128], F32R)
            nc.sync.dma_start(fc3a[:], ap(fc3t, 0, [[128, 128], [128*128, 2], [1, 128]]))
            fc3b = fcp.tile([1, 128], F32R)
            nc.sync.dma_start(fc3b[:], fc3t[256:257, :])
            pf3 = psumF.tile([64, 128], F32, tag="pf3")
            for k in range(2):
                nc.tensor.matmul(pf3[:], f2T[:, k*64:(k+1)*64],
                                 fc3a[:, k*128:(k+1)*128],
                                 start=(k == 0), stop=False)
            nc.tensor.matmul(pf3[:], onesf[0:1, :],
                             fc3b[:], start=False, stop=True)
            fout = fcp.tile([64, 128], F32)
            nc.scalar.activation(fout[:], pf3[:], AF.Identity, bias=zt[0:64, :])
            nc.sync.dma_start(out[:], fout[:])

    nc.compile()
    return nc


# ---------------------------------------------------------------------------
# host side
# ---------------------------------------------------------------------------
def _bf16(x):
    import ml_dtypes
    return np.asarray(x, np.float32).astype(ml_dtypes.bfloat16)


def prep_inputs(x, conv1_w, conv1_b, bn_g, bn_b, pconv_w, pconv_b, W_caps,
                fc1_w, fc1_b, fc2_w, fc2_b, fc3_w, fc3_b):
    x = np.asarray(x, np.float32)
    s_bn = (np.asarray(bn_g) / np.sqrt(1.0 + 1e-5)).astype(np.float32)
    w1f = (np.asarray(conv1_w) * s_bn[:, None, None, None]).astype(np.float32)
    b1v = (np.asarray(conv1_b)*s_bn + np.asarray(bn_b)).astype(np.float32)
    # w1k[27t + ic*9 + ky, bt*256 + oc] = w1f[oc, ic, ky, 4*bt + t]
    w1km = np.zeros((128, 3*256), np.float32)
    for bt in range(3):
        for t in range(4 if bt < 2 else 1):
            kx = 4*bt + t
            blk = np.transpose(w1f[:, :, :, kx], (1, 2, 0)).reshape(27, 256)
            w1km[27*t:27*t+27, bt*256:(bt+1)*256] = blk
    w2t = np.transpose(np.asarray(pconv_w, np.float32), (1, 2, 3, 0))  # [ic,ky,kx,oc]
    w2t = w2t.reshape(2, 128, 9, 9, 2, 128)
    w2m = _bf16(np.ascontiguousarray(
        np.transpose(w2t, (1, 0, 2, 3, 4, 5)).reshape(128, 2*81*2*128)))
    W_caps = np.asarray(W_caps, np.float32)
    # route permutation: idx = kq2*144 + g -> r_local = (idx%32)*18 + idx//32
    idx = np.arange(RPC)
    r_perm = (idx % 32)*18 + idx // 32
    wks = []
    for k in range(N_CORES):
        Wk = W_caps[RPC*k:RPC*(k+1)][r_perm]          # [(kq2,g), ND, PC, DC]
        Wk = Wk.reshape(4, NG, ND, PC, DC)
        wkm = np.transpose(Wk, (0, 3, 1, 2, 4))       # [kq2, c, g, n, o]
        wks.append(_bf16(np.ascontiguousarray(wkm.reshape(128, NG*ND*DC))))
    fc1m = np.concatenate([np.asarray(fc1_w).T, np.asarray(fc1_b)[None, :]], 0).astype(np.float32)
    fc2m = np.concatenate([np.asarray(fc2_w).T, np.asarray(fc2_b)[None, :]], 0).astype(np.float32)
    fc3m = np.concatenate([np.asarray(fc3_w).T, np.asarray(fc3_b)[None, :]], 0).astype(np.float32)
    identm = np.eye(128, dtype=np.float32)
    in_maps = []
    for k in range(N_CORES):
        in_maps.append({
            "xs": np.concatenate([x[BPC*k:BPC*(k+1)].ravel(),
                                  np.zeros(3, np.float32)]),
            "w1k": w1km, "b1": b1v, "w2": w2m,
            "b2": np.asarray(pconv_b, np.float32),
            "wk": wks[k],
            "fc1t": fc1m, "fc2t": fc2m, "fc3t": fc3m,
            "ident": identm, "onesd": np.ones((128, 64), np.float32),
        })
    return in_maps


_NC_CACHE = {}


def kernel(**inputs):
    if 'main' not in _NC_CACHE:
        _NC_CACHE['main'] = build(debug=False)
    nc = _NC_CACHE['main']
    in_maps = prep_inputs(**{k: np.asarray(v) for k, v in inputs.items()})
    res = run_bass_kernel_spmd(nc, in_maps, list(range(N_CORES)))
    return np.asarray(res.results[0]["out"], dtype=np.float32)



# revision 11
# speedup vs baseline: 1.2637x; 1.2637x over previous
"""CapsNet feature extractor on 8 Trainium2 NeuronCores (Bass/Tile). v2

Sharding: phase A (conv1 -> bn/relu -> pconv -> squash) is data-parallel over
batch (8 images/core). An AllToAll reshards u from batch-sharded to
routes-sharded (576 routes/core; core j owns capsule type j). Phase B (u_hat,
dynamic routing) is routes-sharded with an AllReduce on s each iteration; the
final FC stack is computed redundantly on every core; core 0's output is
returned.

v2 changes vs v1:
- conv1 uses a single 108-descriptor full-row input load + kx-batched
  tile_position matmuls (kills the 3MB/image im2col HBM blowup).
- conv1 output stored even/odd-column split (h_eo) so pconv matmul rhs APs
  are stride-1.
- u is transposed on-chip (DVE 32x32 block transpose) before staging, and the
  AllToAll buffers are laid out [dst, kq2, c, img, g] so both the send and
  the post-collective load move >=288B contiguous runs (v1's load moved 2-byte
  runs: ~1.2M descriptors).
- u_hat writeback batched to one DMA per 8-route group (576 -> 72 DMAs).
- W_caps host prep permuted to match the new route->(kq2,g) mapping.
"""
import os
import sys
sys.path.insert(0, '/opt/trn_rl_repo')
import numpy as np
from contextlib import ExitStack

import concourse.bass as bass
import concourse.bacc as bacc
import concourse.mybir as mybir
from concourse import tile
from concourse.bass_utils import run_bass_kernel_spmd

dt = mybir.dt
AF = mybir.ActivationFunctionType
ALU = mybir.AluOpType
AX = mybir.AxisListType

N_CORES = 8
B = 64; IN_C = 3; IMG = 64
NCAP = 8; PC = 32; ND = 10; DC = 16; FEAT = 128
C1 = 56
PR = 24
ROUTES = 4608
BPC = B // N_CORES
RPC = ROUTES // N_CORES
NO = ND * DC                # 160
NG = RPC // 4               # 144
NT = 5                      # u_hat partition tiles (4 x 128 + 1 x 64)
F32, BF16, F32R = dt.float32, dt.bfloat16, dt.float32r


def ap(t, offset, dims):
    """Manual access pattern; dims = [[step, count], ...] outer->inner, elems."""
    if isinstance(t, bass.AP):
        return bass.AP(tensor=t.tensor, offset=t.offset + offset,
                       ap=[list(d) for d in dims])
    return bass.AP(tensor=t, offset=offset, ap=[list(d) for d in dims])


def fap(tl, p0, pcnt, off, dims, pstep_mult=1):
    """AP into an SBUF tile AP `tl`: partition run [p0, p0+pcnt) with partition
    step `pstep_mult` rows, then free dims (offset `off` within partition)."""
    pstep = tl.ap[0][0]
    return bass.AP(tensor=tl.tensor, offset=tl.offset + p0*pstep + off,
                   ap=[[pstep*pstep_mult, pcnt]] + [list(d) for d in dims])


def xap(tl, p0, off, dims):
    """AP into an SBUF tile with explicit (possibly partition-crossing) dims:
    entries of `dims` may be ('p', mult, count) for partition-stepping levels
    or [step, count] for free-dim levels."""
    pstep = tl.ap[0][0]
    out = []
    for d in dims:
        if isinstance(d, tuple) and d[0] == 'p':
            out.append([pstep * d[1], d[2]])
        else:
            out.append([d[0], d[1]])
    return bass.AP(tensor=tl.tensor, offset=tl.offset + p0*pstep + off, ap=out)


def build(debug=False):
    nc = bacc.Bacc("TRN2", target_bir_lowering=False, debug=False,
                   num_devices=N_CORES)

    xs = nc.declare_dram_parameter("xs", [BPC*IN_C*IMG*IMG + 3], F32R, isOutput=False)
    w1k = nc.declare_dram_parameter("w1k", [128, 3*256], F32R, isOutput=False)
    b1 = nc.declare_dram_parameter("b1", [256], F32, isOutput=False)
    w2 = nc.declare_dram_parameter("w2", [128, 2*81*2*128], BF16, isOutput=False)
    b2 = nc.declare_dram_parameter("b2", [256], F32, isOutput=False)
    wk = nc.declare_dram_parameter("wk", [128, NG*NO], BF16, isOutput=False)
    fc1t = nc.declare_dram_parameter("fc1t", [161, 512], F32R, isOutput=False)
    fc2t = nc.declare_dram_parameter("fc2t", [513, 256], F32R, isOutput=False)
    fc3t = nc.declare_dram_parameter("fc3t", [257, 128], F32R, isOutput=False)
    ident = nc.declare_dram_parameter("ident", [128, 128], F32R, isOutput=False)
    onesd = nc.declare_dram_parameter("onesd", [128, 64], F32R, isOutput=False)
    out = nc.declare_dram_parameter("out", [B, FEAT], F32, isOutput=True)

    # A2A staging: flat [dst, kq2*c*img*g]
    u_send = nc.dram_tensor("u_send", [N_CORES, 4*PC*BPC*NG], BF16)
    u_recv = nc.dram_tensor("u_recv", [N_CORES, 4*PC*BPC*NG], BF16)
    # uh bounce: uh-linear [r_local, b, n*o]
    uh_scr = nc.dram_tensor("uh_scr", [RPC, B*NO], BF16)
    s_send = nc.dram_tensor("s_send", [3, B*NO], F32)
    s_recv = nc.dram_tensor("s_recv", [3, B*NO], F32)
    v_stage = nc.dram_tensor("v_stage", [ND, B*DC], F32R)
    groups = [list(range(N_CORES))]

    with tile.TileContext(nc) as tc, ExitStack() as top:
        consts = top.enter_context(tc.tile_pool(name="consts", bufs=1))

        b1t = consts.tile([128, 2], F32)
        nc.sync.dma_start(b1t[:], ap(b1, 0, [[1, 128], [128, 2]]))
        b2t = consts.tile([128, 2], F32)
        nc.sync.dma_start(b2t[:], ap(b2, 0, [[1, 128], [128, 2]]))
        onesb = consts.tile([128, 1], BF16)
        nc.vector.memset(onesb[:], 1.0)
        onesrow = consts.tile([1, 128], BF16)
        nc.vector.memset(onesrow[:], 1.0)
        onesf = consts.tile([128, 64], F32R)
        nc.sync.dma_start(onesf[:], onesd[:])
        identT = consts.tile([128, 128], F32R)
        nc.sync.dma_start(identT[:], ident[:])
        zt = consts.tile([128, 1], F32)
        nc.vector.memset(zt[:], 0.0)
        epst = consts.tile([128, 1], F32)
        nc.vector.memset(epst[:], 1e-8)

        # ============ PHASE A ============
        with tc.tile_pool(name="pha", bufs=1) as pha, \
             tc.tile_pool(name="img", bufs=2) as ipool, \
             tc.tile_pool(name="psumA", bufs=4, space="PSUM") as psumA, \
             tc.tile_pool(name="psumP", bufs=2, space="PSUM") as psumP:
            w1sb = pha.tile([128, 3*256], F32R)
            nc.sync.dma_start(w1sb[:], w1k[:])
            w2sb = pha.tile([128, 2*81*2*128], BF16)
            nc.sync.dma_start(w2sb[:], w2[:])

            for img in range(BPC):
                # --- input rows: partition 32t + ic*9 + ky holds
                #     x[ic, ky:ky+56, 0:64] (3584 elems), replicated t=0..3 ---
                # block t (partitions 27t + 9ic + ky) holds x[ic, ky:ky+56, t:t+64]
                # (kx%4 shift baked into the data) so one plain K=108 matmul
                # covers kx = 4*bt + t for all 4 t's at once.
                xrows = ipool.tile([128, C1*IMG], F32R, tag="xrows")
                for t in range(4):
                    nc.sync.dma_start(
                        xap(xrows, 27*t, 0, [('p', 1, 27), [1, C1*IMG]]),
                        ap(xs, img*IN_C*IMG*IMG + t,
                           [[IMG*IMG, 3], [IMG, 9], [1, C1*IMG]]))

                # --- conv1 (+bn+relu) -> h_eo[p, e*3136 + och*1568 + y*28 + x'] ---
                h_eo = ipool.tile([128, 2*C1*C1], BF16, tag="heo")
                for och in range(2):
                    for oy0 in range(0, 7, 2):
                        noy = min(2, 7 - oy0)
                        pscs = [psumA.tile([128, 448], F32, tag="psc1",
                                           name=f"psc_{och}_{oy0}_{q}")
                                for q in range(noy)]
                        for bt in range(3):
                            lhsT = w1sb[0:108, bt*256 + och*128: bt*256 + (och+1)*128]
                            for q in range(noy):
                                oyc = oy0 + q
                                nc.tensor.matmul(
                                    pscs[q][:],
                                    lhsT,
                                    fap(xrows, 0, 108, oyc*8*IMG + 4*bt,
                                        [[IMG, 8], [1, C1]]),
                                    start=(bt == 0), stop=(bt == 2))
                        for q in range(noy):
                            oyc = oy0 + q
                            nc.scalar.activation(
                                fap(h_eo, 0, 128, och*1568 + oyc*224,
                                    [[C1*C1, 2], [28, 8], [1, 28]]),
                                fap(pscs[q], 0, 128, 0,
                                    [[1, 2], [C1, 8], [2, 28]]),
                                AF.Relu, bias=b1t[:, och:och+1])

                # --- pconv (stride 2) -> pst[p, och*576 + oy*24 + ox] ---
                pst = ipool.tile([128, 2*PR*PR], F32, tag="pst")
                for och in range(2):
                    psA = psumP.tile([128, 288], F32, tag="psA")
                    psB = psumP.tile([128, 288], F32, tag="psB")
                    for ich in range(2):
                        for ky in range(9):
                            for kx in range(9):
                                widx = ((ich*81 + ky*9 + kx)*2 + och)*128
                                lhsT = w2sb[:, widx:widx+128]
                                first = (ich == 0 and ky == 0 and kx == 0)
                                last = (ich == 1 and ky == 8 and kx == 8)
                                e, tt = kx % 2, kx // 2
                                base = e*C1*C1 + ich*1568 + ky*28 + tt
                                nc.tensor.matmul(
                                    psA[:], lhsT,
                                    fap(h_eo, 0, 128, base, [[56, 12], [1, 24]]),
                                    start=first, stop=last)
                                nc.tensor.matmul(
                                    psB[:], lhsT,
                                    fap(h_eo, 0, 128, base + 672, [[56, 12], [1, 24]]),
                                    start=first, stop=last)
                    nc.scalar.activation(pst[:, och*576:och*576+288], psA[:],
                                         AF.Identity, bias=b2t[:, och:och+1])
                    nc.scalar.activation(pst[:, och*576+288:och*576+576], psB[:],
                                         AF.Identity, bias=b2t[:, och:och+1])

                # --- squash over c = q%32 (groups of 32 in free dim) ---
                p2 = ipool.tile([128, 2*PR*PR], F32, tag="p2")
                nc.vector.tensor_mul(p2[:], pst[:], pst[:])
                sn = ipool.tile([128, 36], F32, tag="sn")
                nc.vector.tensor_reduce(sn[:], fap(p2, 0, 128, 0, [[32, 36], [1, 32]]),
                                        AX.X, ALU.add)
                sn1 = ipool.tile([128, 36], F32, tag="sn1")
                nc.vector.tensor_scalar_add(sn1[:], sn[:], 1.0)
                rde = ipool.tile([128, 36], F32, tag="rde")
                nc.vector.reciprocal(rde[:], sn1[:])
                sqr = ipool.tile([128, 36], F32, tag="sqr")
                nc.scalar.activation(sqr[:], sn[:], AF.Sqrt, bias=epst[:, :])
                rsq = ipool.tile([128, 36], F32, tag="rsq")
                nc.vector.reciprocal(rsq[:], sqr[:])
                scl = ipool.tile([128, 36], F32, tag="scl")
                nc.vector.tensor_mul(scl[:], sn[:], rde[:])
                nc.vector.tensor_mul(scl[:], scl[:], rsq[:])
                usq = ipool.tile([128, 2*PR*PR], BF16, tag="usq")
                nc.vector.tensor_mul(usq[:], pst[:],
                                     fap(scl, 0, 128, 0, [[1, 36], [0, 32]]))

                # --- 32x32 block transpose: T[32kq + c, och*576 + qb*32 + pc'] ---
                T_img = ipool.tile([128, 2*PR*PR], BF16, tag="timg")
                nc.vector.transpose(T_img[:], usq[:])

                # --- stage to u_send[j, kq2, c, img, g] (128 desc x 288B) ---
                for j in range(N_CORES):
                    och, kq = j // 4, j % 4
                    nc.sync.dma_start(
                        ap(u_send, j*(4*PC*BPC*NG) + img*NG,
                           [[BPC*NG, 32], [PC*BPC*NG, 4], [1, NG]]),
                        fap(T_img, 32*kq, 32, och*576, [[NG, 4], [1, NG]]))

        nc.gpsimd.collective_compute("AllToAll", ALU.bypass, replica_groups=groups,
                                     ins=[u_send[:]], outs=[u_recv[:]])

        # ============ PHASE B: u_hat ============
        uhp = top.enter_context(tc.tile_pool(name="uhp", bufs=1))
        uh = [uhp.tile([128, B*NO], BF16, name=f"uh{t}", tag=f"uh{t}")
              for t in range(NT)]
        nc.vector.memset(uh[4][64:128, :], 0.0)

        with tc.tile_pool(name="utp", bufs=1) as utp, \
             tc.tile_pool(name="wstr", bufs=2) as wstr, \
             tc.tile_pool(name="stgp", bufs=2) as stgp, \
             tc.tile_pool(name="psumB", bufs=2, space="PSUM") as psumB:
            # u_all[p=(kq2,c), src*1152 + img*144 + g]
            u_all = utp.tile([128, N_CORES*BPC*NG], BF16)
            for src in range(N_CORES):
                nc.sync.dma_start(
                    u_all[:, src*BPC*NG:(src+1)*BPC*NG],
                    ap(u_recv, src*(4*PC*BPC*NG), [[BPC*NG, 128], [1, BPC*NG]]))

            wchunk = None
            stage = None
            for gp in range(NG // 2):
                ck = (2*gp) // 16
                if (2*gp) % 16 == 0:
                    wchunk = wstr.tile([128, 16*NO], BF16, tag="wchunk")
                    nc.sync.dma_start(wchunk[:], wk[:, ck*16*NO:(ck+1)*16*NO])
                if gp % 16 == 0:
                    # wide stage for one uh tile (up to 16 gp = 128 routes)
                    stage = stgp.tile([128, 16*640], BF16, tag="stage")
                ps = psumB.tile([128, 2048], F32, tag="psuh")
                for rt in range(4):
                    for ct, g in ((0, 2*gp), (1, 2*gp+1)):
                        nc.tensor.matmul(
                            ps[64*ct:64*ct+64, rt*512:rt*512+160],
                            fap(u_all, 32*rt, 32, g, [[BPC*NG, 8], [NG, 8]]),
                            wchunk[32*rt:32*rt+32, (g % 16)*NO:(g % 16 + 1)*NO],
                            start=True, stop=True, tile_position=(32*rt, 64*ct))
                pview = fap(ps, 0, 128, 0, [[512, 4], [1, 160]])
                sview = stage[:, (gp % 16)*640:(gp % 16)*640 + 640]
                if gp % 2 == 0:
                    nc.scalar.activation(sview, pview, AF.Identity, bias=zt[:, :])
                else:
                    nc.vector.tensor_copy(sview, pview)
                # stage[p=(ct*64+b), (gpl, rt, no)] -> uh_scr[8*gpl+4*ct+rt, b*160+no]
                # (DRAM bounce: SBUF->SBUF cannot cross partitions on both sides)
                if gp % 16 == 15 or gp == NG//2 - 1:
                    t0 = gp // 16
                    ngp = gp % 16 + 1
                    for ct in range(2):
                        for rt in range(4):
                            eng = nc.sync if (ct*4+rt) % 2 == 0 else nc.scalar
                            eng.dma_start(
                                ap(uh_scr, (t0*128 + 4*ct + rt)*(B*NO),
                                   [[160, 64], [8*B*NO, ngp], [1, 160]]),
                                xap(stage, 64*ct, rt*160,
                                    [('p', 1, 64), [640, ngp], [1, 160]]))
                    eng2 = nc.sync if t0 % 2 == 0 else nc.scalar
                    eng2.dma_start(
                        uh[t0][0:8*ngp, :],
                        ap(uh_scr, t0*128*(B*NO), [[B*NO, 8*ngp], [1, B*NO]]))

        # ============ routing ============
        rp = top.enter_context(tc.tile_pool(name="rp", bufs=1))
        logits = rp.tile([128, NT*B*ND], F32)
        nc.vector.memset(logits[:], 0.0)
        c_t = rp.tile([128, NT*B*ND], BF16)
        s_sb = rp.tile([10, B*DC], F32)
        v_sb = rp.tile([10, B*DC], F32)
        vb16 = rp.tile([10, B*DC], BF16)
        sq2 = rp.tile([10, B*DC], F32)
        snv = rp.tile([10, B], F32)
        snv1 = rp.tile([10, B], F32)
        rdev = rp.tile([10, B], F32)
        sqv = rp.tile([10, B], F32)
        rsqv = rp.tile([10, B], F32)
        sclv = rp.tile([10, B], F32)
        sume = rp.tile([128, NT*B], F32)
        rece = rp.tile([128, NT*B], F32)

        with tc.tile_pool(name="agg", bufs=1) as agg, \
             tc.tile_pool(name="m2cp", bufs=3) as m2cp, \
             tc.tile_pool(name="sstg", bufs=3) as sstg, \
             tc.tile_pool(name="psumR", bufs=2, space="PSUM") as psumR:
            m2b = agg.tile([128, B*NO], BF16)
            l1t = agg.tile([128, B*NO//2], BF16)
            v_rep = agg.tile([128, B*NO], BF16)

            for it in range(3):
                # ---- partial s = sum over local routes of c * u_hat ----
                for chk in range(22):
                    b0 = chk*3
                    nb = min(3, B - b0)
                    w = nb*NO
                    pss = psumR.tile([128, 512], F32, tag="pss")
                    for t in range(NT):
                        kk = 128 if t < 4 else 64
                        if it == 0:
                            rhs = uh[t][0:kk, b0*NO:b0*NO + w]
                        else:
                            m2c = m2cp.tile([128, 512], BF16, tag="m2c")
                            nc.vector.tensor_mul(
                                m2c[0:kk, 0:w], uh[t][0:kk, b0*NO:b0*NO + w],
                                fap(c_t, 0, kk, t*B*ND + b0*ND,
                                    [[ND, nb], [1, ND], [0, DC]]))
                            rhs = m2c[0:kk, 0:w]
                        nc.tensor.matmul(pss[0:1, 0:w], onesb[0:kk, :], rhs,
                                         start=(t == 0), stop=(t == 4))
                    sst = sstg.tile([1, 512], F32, tag="sst")
                    nc.scalar.activation(sst[0:1, 0:w], pss[0:1, 0:w],
                                         AF.Identity, bias=zt[0:1, :],
                                         scale=(0.1 if it == 0 else 1.0))
                    nc.sync.dma_start(ap(s_send, it*B*NO + b0*NO, [[1, w]]),
                                      sst[0:1, 0:w])
                nc.gpsimd.collective_compute(
                    "AllReduce", ALU.add, replica_groups=groups,
                    ins=[ap(s_send, it*B*NO, [[1, B*NO]])],
                    outs=[ap(s_recv, it*B*NO, [[1, B*NO]])])

                # ---- v = squash(s) in [10p=n, (b, o)] ----
                nc.sync.dma_start(s_sb[:],
                                  ap(s_recv, it*B*NO, [[DC, ND], [NO, B], [1, DC]]))
                nc.vector.tensor_mul(sq2[:], s_sb[:], s_sb[:])
                nc.vector.tensor_reduce(snv[:], fap(sq2, 0, 10, 0, [[DC, B], [1, DC]]),
                                        AX.X, ALU.add)
                nc.vector.tensor_scalar_add(snv1[:], snv[:], 1.0)
                nc.vector.reciprocal(rdev[:], snv1[:])
                nc.scalar.activation(sqv[:], snv[:], AF.Sqrt, bias=epst[0:10, :])
                nc.vector.reciprocal(rsqv[:], sqv[:])
                nc.vector.tensor_mul(sclv[:], snv[:], rdev[:])
                nc.vector.tensor_mul(sclv[:], sclv[:], rsqv[:])
                nc.vector.tensor_mul(v_sb[:], s_sb[:],
                                     fap(sclv, 0, 10, 0, [[1, B], [0, DC]]))
                if it == 2:
                    nc.sync.dma_start(v_stage[:].bitcast(F32), v_sb[:])
                    break
                nc.vector.tensor_copy(vb16[:], v_sb[:])
                for n in range(ND):
                    vrow = m2cp.tile([1, B*DC], BF16, tag="vrow")
                    nc.sync.dma_start(vrow[0:1, :], vb16[n:n+1, :])
                    psv = psumR.tile([128, 1024], F32, tag="psv", bufs=1)
                    nc.tensor.matmul(psv[:, 0:512], onesrow[0:1, :],
                                     vrow[0:1, 0:512], start=True, stop=True)
                    nc.tensor.matmul(psv[:, 512:1024], onesrow[0:1, :],
                                     vrow[0:1, 512:1024], start=True, stop=True)
                    nc.scalar.activation(
                        fap(v_rep, 0, 128, n*DC, [[NO, B], [1, DC]]),
                        psv[:], AF.Identity, bias=zt[:, :])

                # ---- agreement: logits += sum_o u_hat * v ----
                for t in range(NT):
                    nc.vector.tensor_mul(m2b[:], uh[t][:], v_rep[:])
                    nc.vector.tensor_add(
                        l1t[:], fap(m2b, 0, 128, 0, [[DC, B*ND], [1, 8]]),
                        fap(m2b, 0, 128, 8, [[DC, B*ND], [1, 8]]))
                    nc.vector.tensor_add(
                        m2b[:, 0:B*NO//4],
                        fap(l1t, 0, 128, 0, [[8, B*ND], [1, 4]]),
                        fap(l1t, 0, 128, 4, [[8, B*ND], [1, 4]]))
                    nc.vector.tensor_add(
                        l1t[:, 0:B*NO//8],
                        fap(m2b, 0, 128, 0, [[4, B*ND], [1, 2]]),
                        fap(m2b, 0, 128, 2, [[4, B*ND], [1, 2]]))
                    nc.vector.tensor_add(
                        m2b[:, 0:B*ND],
                        fap(l1t, 0, 128, 0, [[2, B*ND], [1, 1]]),
                        fap(l1t, 0, 128, 1, [[2, B*ND], [1, 1]]))
                    nc.vector.tensor_add(
                        logits[:, t*B*ND:(t+1)*B*ND],
                        logits[:, t*B*ND:(t+1)*B*ND],
                        m2b[:, 0:B*ND])

                # ---- c = softmax(logits) over n (no max-subtraction) ----
                eexp = m2b[:].bitcast(F32)          # [128, 5120] f32 view
                nc.scalar.activation(eexp[:, 0:NT*B*ND], logits[:], AF.Exp, bias=zt[:, :])
                nc.vector.tensor_reduce(
                    sume[:], fap(eexp, 0, 128, 0, [[ND, NT*B], [1, ND]]),
                    AX.X, ALU.add)
                nc.vector.reciprocal(rece[:], sume[:])
                nc.vector.tensor_mul(c_t[:], eexp[:, 0:NT*B*ND],
                                     fap(rece, 0, 128, 0, [[1, NT*B], [0, ND]]))

        # ============ FC head (redundant on every core) ============
        with tc.tile_pool(name="fcp", bufs=1) as fcp, \
             tc.tile_pool(name="psumF", bufs=1, space="PSUM") as psumF:
            fta = fcp.tile([128, B], F32R)
            ftb = fcp.tile([128, B], F32R)
            for n in range(ND):
                dstt, p0 = (fta, n*DC) if n < 8 else (ftb, (n-8)*DC)
                nc.sync.dma_start(dstt[p0:p0+DC, :],
                                  ap(v_stage, n*B*DC, [[1, DC], [DC, B]]))
            nc.sync.dma_start(ftb[32:33, :], onesd[0:1, :])

            fc1a = fcp.tile([128, 512], F32R)
            nc.sync.dma_start(fc1a[:], fc1t[0:128, :])
            fc1b = fcp.tile([128, 512], F32R)
            nc.sync.dma_start(fc1b[0:33, :], fc1t[128:161, :])
            pf1 = psumF.tile([64, 512], F32, tag="pf1")
            nc.tensor.matmul(pf1[:], fta[:, 0:64],
                             fc1a[:], start=True, stop=False)
            nc.tensor.matmul(pf1[:], ftb[0:33, 0:64],
                             fc1b[0:33, :], start=False, stop=True)
            f1 = fcp.tile([64, 512], F32R)
            nc.scalar.activation(f1[:], pf1[:], AF.Relu, bias=zt[0:64, :])

            f1T = fcp.tile([128, 4*64], F32R)
            for k in range(4):
                ptr = psumF.tile([128, 64], F32R, tag="ptr", bufs=2)
                nc.tensor.transpose(ptr[:], f1[:, k*128:(k+1)*128], identT[0:64, 0:64])
                nc.scalar.activation(f1T[:, k*64:(k+1)*64], ptr[:], AF.Identity, bias=zt[:, :])

            fc2a = fcp.tile([128, 4*256], F32R)
            nc.sync.dma_start(fc2a[:], ap(fc2t, 0, [[256, 128], [128*256, 4], [1, 256]]))
            fc2b = fcp.tile([1, 256], F32R)
            nc.sync.dma_start(fc2b[:], fc2t[512:513, :])
            pf2 = psumF.tile([64, 256], F32, tag="pf2")
            for k in range(4):
                nc.tensor.matmul(pf2[:], f1T[:, k*64:(k+1)*64],
                                 fc2a[:, k*256:(k+1)*256],
                                 start=(k == 0), stop=False)
            nc.tensor.matmul(pf2[:], onesf[0:1, :],
                             fc2b[:], start=False, stop=True)
            f2 = fcp.tile([64, 256], F32R)
            nc.scalar.activation(f2[:], pf2[:], AF.Relu, bias=zt[0:64, :])

            f2T = fcp.tile([128, 2*64], F32R)
            for k in range(2):
                ptr2 = psumF.tile([128, 64], F32R, tag="ptr", bufs=2)
                nc.tensor.transpose(ptr2[:], f2[:, k*128:(k+1)*128], identT[0:64, 0:64])
                nc.scalar.activation(f2T[:, k*64:(k+1)*64], ptr2[:], AF.Identity, bias=zt[:, :])

            fc3a = fcp.tile([128, 2*128], F32R)
            nc.sync.dma_start(fc3a[:], ap(fc3t, 0, [[128, 128], [128*128, 2], [1, 128]]))
            fc3b = fcp.tile([1, 128], F32R)
            nc.sync.dma_start(fc3b[:], fc3t[256:257, :])
            pf3 = psumF.tile([64, 128], F32, tag="pf3")
            for k in range(2):
                nc.tensor.matmul(pf3[:], f2T[:, k*64:(k+1)*64],
                                 fc3a[:, k*128:(k+1)*128],
                                 start=(k == 0), stop=False)
            nc.tensor.matmul(pf3[:], onesf[0:1, :],
                             fc3b[:], start=False, stop=True)
            fout = fcp.tile([64, 128], F32)
            nc.scalar.activation(fout[:], pf3[:], AF.Identity, bias=zt[0:64, :])
            nc.sync.dma_start(out[:], fout[:])

    nc.compile()
    return nc


# ---------------------------------------------------------------------------
# host side
# ---------------------------------------------------------------------------
def _bf16(x):
    import ml_dtypes
    return np.asarray(x, np.float32).astype(ml_dtypes.bfloat16)


def prep_inputs(x, conv1_w, conv1_b, bn_g, bn_b, pconv_w, pconv_b, W_caps,
                fc1_w, fc1_b, fc2_w, fc2_b, fc3_w, fc3_b):
    x = np.asarray(x, np.float32)
    s_bn = (np.asarray(bn_g) / np.sqrt(1.0 + 1e-5)).astype(np.float32)
    w1f = (np.asarray(conv1_w) * s_bn[:, None, None, None]).astype(np.float32)
    b1v = (np.asarray(conv1_b)*s_bn + np.asarray(bn_b)).astype(np.float32)
    # w1k[27t + ic*9 + ky, bt*256 + oc] = w1f[oc, ic, ky, 4*bt + t]
    w1km = np.zeros((128, 3*256), np.float32)
    for bt in range(3):
        for t in range(4 if bt < 2 else 1):
            kx = 4*bt + t
            blk = np.transpose(w1f[:, :, :, kx], (1, 2, 0)).reshape(27, 256)
            w1km[27*t:27*t+27, bt*256:(bt+1)*256] = blk
    w2t = np.transpose(np.asarray(pconv_w, np.float32), (1, 2, 3, 0))  # [ic,ky,kx,oc]
    w2t = w2t.reshape(2, 128, 9, 9, 2, 128)
    w2m = _bf16(np.ascontiguousarray(
        np.transpose(w2t, (1, 0, 2, 3, 4, 5)).reshape(128, 2*81*2*128)))
    W_caps = np.asarray(W_caps, np.float32)
    # route permutation: idx = kq2*144 + g -> r_local = (idx%32)*18 + idx//32
    idx = np.arange(RPC)
    r_perm = (idx % 32)*18 + idx // 32
    wks = []
    for k in range(N_CORES):
        Wk = W_caps[RPC*k:RPC*(k+1)][r_perm]          # [(kq2,g), ND, PC, DC]
        Wk = Wk.reshape(4, NG, ND, PC, DC)
        wkm = np.transpose(Wk, (0, 3, 1, 2, 4))       # [kq2, c, g, n, o]
        wks.append(_bf16(np.ascontiguousarray(wkm.reshape(128, NG*ND*DC))))
    fc1m = np.concatenate([np.asarray(fc1_w).T, np.asarray(fc1_b)[None, :]], 0).astype(np.float32)
    fc2m = np.concatenate([np.asarray(fc2_w).T, np.asarray(fc2_b)[None, :]], 0).astype(np.float32)
    fc3m = np.concatenate([np.asarray(fc3_w).T, np.asarray(fc3_b)[None, :]], 0).astype(np.float32)
    identm = np.eye(128, dtype=np.float32)
    in_maps = []
    for k in range(N_CORES):
        in_maps.append({
            "xs": np.concatenate([x[BPC*k:BPC*(k+1)].ravel(),
                                  np.zeros(3, np.float32)]),
            "w1k": w1km, "b1": b1v, "w2": w2m,
            "b2": np.asarray(pconv_b, np.float32),
            "wk": wks[k],
            "fc1t": fc1m, "fc2t": fc2m, "fc3t": fc3m,
            "ident": identm, "onesd": np.ones((128, 64), np.float32),
        })
    return in_maps


_NC_CACHE = {}


def kernel(**inputs):
    if 'main' not in _NC_CACHE:
        _NC_CACHE['main'] = build(debug=False)
    nc = _NC_CACHE['main']
    in_maps = prep_inputs(**{k: np.asarray(v) for k, v in inputs.items()})
    res = run_bass_kernel_spmd(nc, in_maps, list(range(N_CORES)))
    return np.asarray(res.results[0]["out"], dtype=np.float32)

